# revision 4
# baseline (speedup 1.0000x reference)
"""CondAttnBlock Trainium2 kernel: GN -> attention -> proj -> residual.

Data-parallel over B=32 across 8 NeuronCores (4 batches/core), weights
replicated, no collectives.  Measured rel L2 error vs fp32 jax reference:
1.04e-4.  TimelineSim full-program estimate: 115.2us (v1 baseline: 168.3us,
-32%); steady-state marginal ~74us/invocation (v1: 128.1us, -42%).

Key structure (vs the v1 q/k-fold kernel):
  * Transposed score layout S^T[m,s]: softmax-normalized P feeds the output
    matmul with NO per-batch PE transposes; the rank-1 t[m] term folds into
    the ACT exp bias (per-partition).
  * Row-sums of exp via PE ones-contraction; normalization deferred past the
    output matmul: out = x + (W2'^T P)*rinv_bcast, rinv broadcast to 128
    partitions by a K=1 PE matmul.  bv/bp fold into W2' so they survive the
    deferred normalization exactly.
  * W2 computed per batch directly from yT via precomputed
    WVP[d,o] = sum_c wv[c,d] wp[o,c]: drops the per-batch vT pass.
  * GroupNorm stats via DVE bn_stats/bn_aggr (one pass, no ACT Square).
  * Elementwise spread across DVE/ACT/Pool(gpsimd); emission order places
    next-batch PE work inside the exp->recip->bcast latency windows.
  * Startup: W1T/WVP accumulate chunk-by-chunk as weight DMAs land.
"""

import sys

if "/opt/trn_rl_repo" not in sys.path:
    sys.path.insert(0, "/opt/trn_rl_repo")

from contextlib import ExitStack

import numpy as np

import concourse.bacc as bacc
import concourse.bass as bass
import concourse.mybir as mybir
import concourse.tile as tile

F32 = mybir.dt.float32
F32R = mybir.dt.float32r
I32 = mybir.dt.int32
AF = mybir.ActivationFunctionType
ALU = mybir.AluOpType
AX = mybir.AxisListType

B, C, S, M, D = 32, 512, 1024, 256, 768
G, CPG = 32, 16
NCORES = 8
BPC = B // NCORES
NCH = C // 128  # 4
NDH = D // 128  # 6
NMH = M // 128  # 2
EPS = 1e-5
ATT_SCALE = float(C) ** -0.5
MAGIC = 0x5F3759DF


def r(ap):
    return ap.bitcast(F32R)


def build_program(reps=1):
    nc = bacc.Bacc("TRN2", target_bir_lowering=False, debug=False)

    x_d = nc.dram_tensor("x", [BPC, C, S], F32, kind="ExternalInput").ap()
    y_d = nc.dram_tensor("y", [BPC, M, D], F32, kind="ExternalInput").ap()
    wq_d = nc.dram_tensor("wq", [C, C], F32, kind="ExternalInput").ap()
    wk_d = nc.dram_tensor("wk", [C, D], F32, kind="ExternalInput").ap()
    wv_d = nc.dram_tensor("wv", [C, D], F32, kind="ExternalInput").ap()
    wp_d = nc.dram_tensor("wp", [C, C], F32, kind="ExternalInput").ap()
    bq_d = nc.dram_tensor("bq", [C], F32, kind="ExternalInput").ap()
    bk_d = nc.dram_tensor("bk", [C], F32, kind="ExternalInput").ap()
    bv_d = nc.dram_tensor("bv", [C], F32, kind="ExternalInput").ap()
    bp_d = nc.dram_tensor("bp", [C], F32, kind="ExternalInput").ap()
    gns_d = nc.dram_tensor("gn_scale", [C], F32, kind="ExternalInput").ap()
    gnb_d = nc.dram_tensor("gn_bias", [C], F32, kind="ExternalInput").ap()
    eye_d = nc.dram_tensor("eye", [128, 128], F32, kind="ExternalInput").ap()
    ones_d = nc.dram_tensor("ones", [1, S], F32, kind="ExternalInput").ap()
    onescol_d = nc.dram_tensor("onescol", [128, 1], F32, kind="ExternalInput").ap()
    gmap_d = nc.dram_tensor("gmap", [C, G], F32, kind="ExternalInput").ap()
    gmapT_d = nc.dram_tensor("gmapT", [G, C], F32, kind="ExternalInput").ap()
    out_d = nc.dram_tensor("out", [BPC, C, S], F32, kind="ExternalOutput").ap()

    with tile.TileContext(nc) as tc, ExitStack() as ctx:
        wpool = ctx.enter_context(tc.tile_pool(name="w", bufs=1))
        xpool = ctx.enter_context(tc.tile_pool(name="x", bufs=3))
        ybpool = ctx.enter_context(tc.tile_pool(name="ybp", bufs=4))
        ypool = ctx.enter_context(tc.tile_pool(name="yT", bufs=2))
        rapool = ctx.enter_context(tc.tile_pool(name="ra", bufs=2))
        w2pool = ctx.enter_context(tc.tile_pool(name="w2", bufs=2))
        ppool = ctx.enter_context(tc.tile_pool(name="p", bufs=6))
        spool = ctx.enter_context(tc.tile_pool(name="st", bufs=4))
        otpool = ctx.enter_context(tc.tile_pool(name="ot", bufs=5))
        # PSUM budget (8 banks): scps 2 + outps 3 + repps 2 + miscps 1.
        # Single tag per pool so all shapes share the same buffers.
        scps = ctx.enter_context(tc.tile_pool(name="scps", bufs=2, space="PSUM"))
        outps = ctx.enter_context(tc.tile_pool(name="outps", bufs=3, space="PSUM"))
        repps = ctx.enter_context(tc.tile_pool(name="repps", bufs=2, space="PSUM"))
        miscps = ctx.enter_context(tc.tile_pool(name="miscps", bufs=1, space="PSUM"))

        def sc_ps(shape):
            return scps.tile(shape, F32, tag="ps", name="scps")

        def out_ps(shape):
            return outps.tile(shape, F32, tag="ps", name="outps")

        def rep_ps(shape):
            return repps.tile(shape, F32, tag="ps", name="repps")

        def misc_ps(shape):
            return miscps.tile(shape, F32, tag="ps", name="miscps")

        batch_seq = [bb for _ in range(reps) for bb in range(BPC)]
        NB = len(batch_seq)

        # ---------------- persistent weight/constant tiles ----------------
        eye_r = wpool.tile([128, 128], F32, tag="eyer")
        W1T = wpool.tile([128, NDH * C], F32, tag="W1T")  # [d, c'] chunks
        WVP = wpool.tile([128, NDH * C], F32, tag="WVP")  # [d, o] chunks
        wpT = wpool.tile([128, NCH * C], F32, tag="wpT")  # [c, o] chunks
        wqbk_row = wpool.tile([1, C], F32, tag="wqbk")
        bqwk_col = wpool.tile([128, NDH], F32, tag="bqwk")  # pre-scaled
        bpv_row = wpool.tile([1, C], F32, tag="bpv")  # bvwp + bp
        ones_sb = wpool.tile([1, M], F32, tag="ones")
        onescol_sb = wpool.tile([128, 1], F32, tag="onescol")
        gmap_sb = wpool.tile([128, NCH * G], F32, tag="gmap")
        gmapT_sb = wpool.tile([G, C], F32, tag="gmapT")
        gns_col = wpool.tile([128, NCH], F32, tag="gns")
        gnb_col = wpool.tile([128, NCH], F32, tag="gnb")

        # ---------------- DMA helpers ----------------
        def load_x(b):
            """x[b] as [128, NCH*S]; two DMAs so stats can start early."""
            xt = xpool.tile([128, NCH * S], F32, tag="xb")
            dst = xt[:].rearrange("p (n f) -> p n f", n=NCH)
            src = x_d[b].rearrange("(n p) f -> p n f", p=128)
            nc.sync.dma_start(r(dst[:, 0:2]), r(src[:, 0:2]))
            nc.sync.dma_start(r(dst[:, 2:4]), r(src[:, 2:4]))
            return xt

        def load_y(b):
            yt_ = ybpool.tile([128, NMH * D], F32, tag="yb")
            dst = yt_[:].rearrange("p (n f) -> p n f", n=NMH)
            src = y_d[b].rearrange("(n p) f -> p n f", p=128)
            nc.sync.dma_start(r(dst), r(src))
            return yt_

        # ---------------- emission helpers ----------------
        def alloc_yT():
            return ypool.tile([128, NDH * M], F32, tag="yT", name="yT")

        def emit_yT_part(yT, yb, dis, ps_alloc, copy_eng="act"):
            """Transpose y chunks di in `dis` into yT; copies on ACT/Pool."""
            for di in dis:
                pt = ps_alloc([128, M])
                for mj in range(NMH):
                    nc.tensor.matmul(
                        r(pt[:, mj * 128 : (mj + 1) * 128]),
                        lhsT=r(yb[:, mj * D + di * 128 : mj * D + (di + 1) * 128]),
                        rhs=r(eye_r[:]),
                        is_transpose=True,
                        start=(mj == 0),
                        stop=(mj == NMH - 1),
                    )
                if copy_eng == "act":
                    nc.scalar.copy(r(yT[:, di * M : (di + 1) * M]), pt[:])
                else:
                    nc.vector.tensor_copy(r(yT[:, di * M : (di + 1) * M]), pt[:])

        def emit_stats_p1(xb, cis):
            """bn_stats/aggr for chunks in cis -> per-channel (mean, E[x^2])."""
            outs = []
            for ci in cis:
                bnraw = spool.tile([128, 2 * 6], F32, tag=f"bnraw{ci}", name="bnraw")
                for h in range(2):
                    nc.vector.bn_stats(
                        bnraw[:, h * 6 : (h + 1) * 6],
                        xb[:, ci * S + h * 512 : ci * S + (h + 1) * 512],
                    )
                mv = spool.tile([128, 2], F32, tag=f"mv{ci}", name="mv")
                nc.vector.bn_aggr(mv[:], bnraw[:])
                outs.append(mv)
            return outs

        def emit_stats_stat2(mvs):
            """Combine per-chunk (mean, var) -> stat2 [128, 2*NCH]."""
            stat2 = spool.tile([128, 2 * NCH], F32, tag="stat2")
            for ci in range(NCH):
                nc.vector.tensor_copy(stat2[:, 2 * ci : 2 * ci + 1], mvs[ci][:, 0:1])
            sq = spool.tile([128, NCH], F32, tag="sqm")
            for ci in range(NCH):
                nc.vector.tensor_mul(sq[:, ci : ci + 1], mvs[ci][:, 0:1], mvs[ci][:, 0:1])
                nc.vector.tensor_add(
                    stat2[:, 2 * ci + 1 : 2 * ci + 2], mvs[ci][:, 1:2], sq[:, ci : ci + 1]
                )
            return stat2

        def emit_stats_gnmm(stat2):
            """PE: group aggregate; DVE: scale + Newton rsqrt -> bstat."""
            gps = misc_ps([G, 2])
            for ci in range(NCH):
                nc.tensor.matmul(
                    gps[:],
                    lhsT=gmap_sb[:, ci * G : (ci + 1) * G],
                    rhs=stat2[:, 2 * ci : 2 * ci + 2],
                    start=(ci == 0),
                    stop=(ci == NCH - 1),
                )
            gstat = spool.tile([G, 2], F32, tag="gstat")
            nc.vector.tensor_scalar_mul(gstat[:], gps[:], 1.0 / CPG)
            msq = spool.tile([G, 1], F32, tag="msq")
            nc.vector.tensor_mul(msq[:], gstat[:, 0:1], gstat[:, 0:1])
            veps = spool.tile([G, 1], F32, tag="veps")
            nc.vector.scalar_tensor_tensor(
                veps[:], in0=msq[:], scalar=-1.0, in1=gstat[:, 1:2], op0=ALU.mult, op1=ALU.add
            )
            nc.vector.tensor_scalar_add(veps[:], veps[:], EPS)
            yk = spool.tile([G, 1], F32, tag="yk")
            nc.vector.tensor_scalar(
                yk[:].bitcast(I32), veps[:].bitcast(I32), 1, None, op0=ALU.logical_shift_right
            )
            nc.vector.tensor_scalar(
                yk[:].bitcast(I32), yk[:].bitcast(I32), MAGIC + 1, None, op0=ALU.subtract
            )
            nc.vector.tensor_scalar(
                yk[:].bitcast(I32), yk[:].bitcast(I32), -1, None, op0=ALU.bitwise_xor
            )
            for _ in range(2):
                y2 = spool.tile([G, 1], F32, tag="y2")
                nc.vector.tensor_mul(y2[:], yk[:], yk[:])
                nc.vector.tensor_mul(y2[:], y2[:], veps[:])
                nc.vector.tensor_scalar(y2[:], y2[:], -0.5, 1.5, op0=ALU.mult, op1=ALU.add)
                nc.vector.tensor_mul(yk[:], yk[:], y2[:])
            bstat = spool.tile([G, 2], F32, tag="bstat")  # (mean, rstd)
            nc.vector.tensor_copy(bstat[:, 0:1], gstat[:, 0:1])
            nc.vector.tensor_copy(bstat[:, 1:2], yk[:])
            return bstat

        def emit_stats_chan(bstat):
            """PE channel expand (one psum) + all-DVE tail -> (a, e*scale)."""
            cps = misc_ps([128, 2 * NCH])
            for ci in range(NCH):
                nc.tensor.matmul(
                    cps[:, 2 * ci : 2 * ci + 2],
                    lhsT=gmapT_sb[:, ci * 128 : (ci + 1) * 128],
                    rhs=bstat[:],
                    start=True,
                    stop=True,
                )
            chan = spool.tile([128, 2 * NCH], F32, tag="chan")
            nc.vector.tensor_copy(chan[:], cps[:])
            a_col = spool.tile([128, NCH], F32, tag="acol")
            nc.vector.tensor_mul(a_col[:], chan[:, 1 : 2 * NCH : 2], gns_col[:])
            ra_col = spool.tile([128, NCH], F32, tag="racol")
            nc.vector.reciprocal(ra_col[:], a_col[:])
            etmp = spool.tile([128, NCH], F32, tag="etmp")
            nc.vector.tensor_mul(etmp[:], gnb_col[:], ra_col[:])
            nc.vector.tensor_sub(etmp[:], etmp[:], chan[:, 0 : 2 * NCH : 2])
            e_col = spool.tile([128, NCH], F32, tag="ecol")
            nc.vector.tensor_scalar_mul(r(e_col[:]), etmp[:], ATT_SCALE)
            return a_col, e_col

        def emit_R_mm(yT, cj, ps_alloc=None):
            """Unscaled R chunk cj in PSUM: sum_d W1T[d,c] yT[d,m] + wqbk[c]."""
            ps = (ps_alloc or sc_ps)([128, M])
            for di in range(NDH):
                nc.tensor.matmul(
                    ps[:],
                    lhsT=r(W1T[:, di * C + cj * 128 : di * C + (cj + 1) * 128]),
                    rhs=r(yT[:, di * M : (di + 1) * M]),
                    start=(di == 0),
                    stop=False,
                )
            nc.tensor.matmul(
                ps[:],
                lhsT=r(wqbk_row[:, cj * 128 : (cj + 1) * 128]),
                rhs=r(ones_sb[:, 0:M]),
                start=False,
                stop=True,
            )
            return ps

        def emit_Ra(yT, a_col):
            """Ra[c,m] = a_c * (sum_d W1T[d,c] yT[d,m] + wqbk[c])."""
            Ra = rapool.tile([128, NCH * M], F32, tag="Ra")
            for cj in range(NCH):
                ps = emit_R_mm(yT, cj)
                nc.vector.tensor_scalar_mul(
                    r(Ra[:, cj * M : (cj + 1) * M]), ps[:], a_col[:, cj : cj + 1]
                )
            return Ra

        def emit_t(yT, Ra, e_col):
            """t2col [128, NMH]: per-partition exp bias (pre-scaled)."""
            tps = misc_ps([1, M])
            for cj in range(NCH):
                nc.tensor.matmul(
                    tps[:],
                    lhsT=r(e_col[:, cj : cj + 1]),
                    rhs=r(Ra[:, cj * M : (cj + 1) * M]),
                    start=(cj == 0),
                    stop=False,
                )
            for di in range(NDH):
                nc.tensor.matmul(
                    tps[:],
                    lhsT=r(bqwk_col[:, di : di + 1]),
                    rhs=r(yT[:, di * M : (di + 1) * M]),
                    start=False,
                    stop=(di == NDH - 1),
                )
            t_row = spool.tile([1, M], F32, tag="trow")
            nc.scalar.copy(t_row[:], tps[:])
            tcps = misc_ps([128, NMH])
            for mj in range(NMH):
                # transpose row->column via K=1 matmul against scalar one
                nc.tensor.matmul(
                    tcps[:, mj : mj + 1],
                    lhsT=t_row[:, mj * 128 : (mj + 1) * 128],
                    rhs=onescol_sb[0:1, 0:1],
                    start=True,
                    stop=True,
                )
            t2col = spool.tile([128, NMH], F32, tag="t2col")
            nc.vector.tensor_copy(t2col[:], tcps[:])
            return t2col

        def emit_W2_mj(W2, yT, mj, ps_alloc=None):
            """W2'[m,o] = sum_d yT[d,m] WVP[d,o] + (bvwp+bp)[o], one m-chunk."""
            ps = (ps_alloc or misc_ps)([128, C])
            for di in range(NDH):
                nc.tensor.matmul(
                    ps[:],
                    lhsT=r(yT[:, di * M + mj * 128 : di * M + (mj + 1) * 128]),
                    rhs=r(WVP[:, di * C : (di + 1) * C]),
                    start=(di == 0),
                    stop=False,
                )
            nc.tensor.matmul(
                ps[:],
                lhsT=r(ones_sb[:, 0:128]),
                rhs=r(bpv_row[:]),
                start=False,
                stop=True,
            )
            nc.scalar.copy(r(W2[:, mj * C : (mj + 1) * C]), ps[:])

        # ---------------- startup ----------------
        ys, xs = {}, {}
        yTs, mvs_, stat2s, bstats, abes, Ras, t2s, W2s = {}, {}, {}, {}, {}, {}, {}, {}

        def load_x4(b):
            """x[b] as four per-chunk DMAs (streaming GN stats)."""
            xt = xpool.tile([128, NCH * S], F32, tag="xb", name="xb")
            dst = xt[:].rearrange("p (n f) -> p n f", n=NCH)
            src = x_d[b].rearrange("(n p) f -> p n f", p=128)
            for cj in range(NCH):
                nc.sync.dma_start(r(dst[:, cj : cj + 1]), r(src[:, cj : cj + 1]))
            return xt

        wnat = ctx.enter_context(tc.tile_pool(name="wnat", bufs=1))

        # Alternate HWDGE rings (SP / ACT) so descriptor generation is not
        # the startup bottleneck.
        _ring = [0]

        def dma2(dst, src_):
            # ACT assists descriptor generation only for the first few DMAs;
            # later issues would block ACT compute behind the dma-issue queue.
            eng = nc.scalar if (_ring[0] % 2 == 1 and _ring[0] < 12) else nc.sync
            _ring[0] += 1
            eng.dma_start(dst, src_)

        # ---- DMA priority order ----
        ys[0] = load_y(batch_seq[0])
        dma2(r(eye_r[:]), r(eye_d[:]))
        dma2(r(onescol_sb[:]), r(onescol_d[:]))
        wk_nat = wnat.tile([128, NCH * D], F32, tag="wk_nat")
        wq_sb = wnat.tile([128, NCH * C], F32, tag="wq_nat")
        wkc = wk_nat[:].rearrange("p (n f) -> p n f", n=NCH)
        wks = wk_d.rearrange("(n p) f -> p n f", p=128)
        wqc = wq_sb[:].rearrange("p (n f) -> p n f", n=NCH)
        wqs = wq_d.rearrange("(n p) f -> p n f", p=128)
        for cj in range(NCH):
            dma2(r(wkc[:, cj : cj + 1]), r(wks[:, cj : cj + 1]))
            dma2(r(wqc[:, cj : cj + 1]), r(wqs[:, cj : cj + 1]))
        ys[1] = load_y(batch_seq[1])
        # smalls needed by wqbk/bqwk/R0/rs
        bq2 = wnat.tile([128, 2 * NCH], F32, tag="bq2")
        dma2(r(bq2[:, 0 : 2 * NCH : 2]), r(bq_d.rearrange("(n p) -> p n", p=128)))
        dma2(r(bq2[:, 1 : 2 * NCH : 2]), r(bq_d.rearrange("(n p) -> p n", p=128)))
        bk_col = wnat.tile([128, NCH], F32, tag="bk_col")
        dma2(r(bk_col[:]), r(bk_d.rearrange("(n p) -> p n", p=128)))
        dma2(r(ones_sb[:]), r(ones_d[:, 0:M]))
        # x0 chunked so GN stats stream behind the DMA
        xt0 = xpool.tile([128, NCH * S], F32, tag="xb", name="xb")
        dst0 = xt0[:].rearrange("p (n f) -> p n f", n=NCH)
        src0 = x_d[batch_seq[0]].rearrange("(n p) f -> p n f", p=128)
        for cj in range(NCH):
            dma2(r(dst0[:, cj : cj + 1]), r(src0[:, cj : cj + 1]))
        xs[0] = xt0
        # smalls needed by the GN stat aggregation
        dma2(gmap_sb[:].rearrange("p (n f) -> p n f", n=NCH),
             gmap_d.rearrange("(n p) f -> p n f", p=128))
        dma2(gmapT_sb[:], gmapT_d[:])
        dma2(gns_col[:], gns_d.rearrange("(n p) -> p n", p=128))
        dma2(gnb_col[:], gnb_d.rearrange("(n p) -> p n", p=128))
        wp_nat = wnat.tile([128, NCH * C], F32, tag="wp_nat")
        dma2(
            r(wp_nat[:].rearrange("p (n f) -> p n f", n=NCH)),
            r(wp_d.rearrange("(n p) f -> p n f", p=128)),
        )
        wv_nat = wnat.tile([128, NCH * D], F32, tag="wk_nat", name="wv_nat")
        wvc = wv_nat[:].rearrange("p (n f) -> p n f", n=NCH)
        wvs = wv_d.rearrange("(n p) f -> p n f", p=128)
        for cj in range(NCH):
            dma2(r(wvc[:, cj : cj + 1]), r(wvs[:, cj : cj + 1]))
        bv_col = wnat.tile([128, NCH], F32, tag="bv_col")
        dma2(r(bv_col[:]), r(bv_d.rearrange("(n p) -> p n", p=128)))
        bp_row = wnat.tile([1, C], F32, tag="bp_row")
        dma2(r(bp_row[:]), r(bp_d.rearrange("(a c) -> a c", a=1)))
        xs[1] = load_x(batch_seq[1])
        if NB > 2:
            ys[2] = load_y(batch_seq[2])
            xs[2] = load_x(batch_seq[2])

        # ---- prologue compute ----
        # yT0 as soon as y0+eye land (DVE copies: ACT seq is busy issuing DMAs)
        yTs[0] = alloc_yT()
        emit_yT_part(yTs[0], ys[0], range(NDH), out_ps, "dve")

        # W1T[d,c'] = sum_c wk[c,d] wq[c,c'] (6 live banks, per-chunk accum)
        w1ps = [sc_ps([128, C]) for _ in range(2)]
        w1ps += [out_ps([128, C]) for _ in range(3)]
        w1ps += [rep_ps([128, C])]
        for cj in range(NCH):
            for di in range(NDH):
                nc.tensor.matmul(
                    w1ps[di][:],
                    lhsT=r(wk_nat[:, cj * D + di * 128 : cj * D + (di + 1) * 128]),
                    rhs=r(wq_sb[:, cj * C : (cj + 1) * C]),
                    start=(cj == 0),
                    stop=(cj == NCH - 1),
                )
        for di in range(NDH):
            if di % 2 == 0:
                nc.scalar.copy(r(W1T[:, di * C : (di + 1) * C]), w1ps[di][:])
            else:
                nc.vector.tensor_copy(r(W1T[:, di * C : (di + 1) * C]), w1ps[di][:])

        # wqbk[c'] = sum_c wq[c,c'] bk[c]
        ps = misc_ps([1, C])
        for cj in range(NCH):
            nc.tensor.matmul(
                ps[:],
                lhsT=r(bk_col[:, cj : cj + 1]),
                rhs=r(wq_sb[:, cj * C : (cj + 1) * C]),
                start=(cj == 0),
                stop=(cj == NCH - 1),
            )
        nc.scalar.copy(r(wqbk_row[:]), ps[:])
        # bqwk[d] = ATT_SCALE * sum_c bq[c] wk[c,d]
        for di in range(NDH):
            ps = misc_ps([128, 2])
            for cj in range(NCH):
                nc.tensor.matmul(
                    ps[:],
                    lhsT=r(wk_nat[:, cj * D + di * 128 : cj * D + (di + 1) * 128]),
                    rhs=r(bq2[:, 2 * cj : 2 * cj + 2]),
                    start=(cj == 0),
                    stop=(cj == NCH - 1),
                )
            nc.vector.tensor_scalar_mul(r(bqwk_col[:, di : di + 1]), ps[:, 0:1], ATT_SCALE)

        # stats0 (DVE; streams behind the chunked x0 DMAs)
        mvs_[0] = emit_stats_p1(xs[0], range(NCH))
        stat2s[0] = emit_stats_stat2(mvs_[0])

        # unscaled R(0) chunks fill the PE while stats0 finish
        Rps0 = [emit_R_mm(yTs[0], cj, [sc_ps, sc_ps, out_ps, out_ps][cj]) for cj in range(NCH)]

        # yT(1): PE transposes fill the Newton window; psums via the idle
        # rep pool (sc/out are holding R(0)); copies on ACT (idle by now)
        yTs[1] = alloc_yT()
        emit_yT_part(yTs[1], ys[1], range(0, 4), rep_ps, "act")

        bstats[0] = emit_stats_gnmm(stat2s[0])
        emit_yT_part(yTs[1], ys[1], range(4, NDH), rep_ps, "act")
        abes[0] = emit_stats_chan(bstats[0])
        Ras[0] = rapool.tile([128, NCH * M], F32, tag="Ra", name="Ra")
        for cj in range(NCH):
            nc.vector.tensor_scalar_mul(
                r(Ras[0][:, cj * M : (cj + 1) * M]), Rps0[cj][:], abes[0][0][:, cj : cj + 1]
            )
        t2s[0] = None

        # wpT via PE transpose (wp arrives late; only WVP consumes it)
        for ci in range(NCH):
            pt = misc_ps([128, C])
            for oj in range(NCH):
                nc.tensor.matmul(
                    r(pt[:, oj * 128 : (oj + 1) * 128]),
                    lhsT=r(wp_nat[:, oj * C + ci * 128 : oj * C + (ci + 1) * 128]),
                    rhs=r(eye_r[:]),
                    is_transpose=True,
                    start=(oj == 0),
                    stop=(oj == NCH - 1),
                )
            nc.scalar.copy(r(wpT[:, ci * C : (ci + 1) * C]), pt[:])

        W2s[0] = w2pool.tile([128, NMH * C], F32, tag="W2", name="W2")

        def emit_WVP():
            # bpv[o] = sum_c bv[c] wp[o,c] + bp[o]
            ps = misc_ps([1, C])
            for cj in range(NCH):
                nc.tensor.matmul(
                    ps[:],
                    lhsT=r(bv_col[:, cj : cj + 1]),
                    rhs=r(wpT[:, cj * C : (cj + 1) * C]),
                    start=(cj == 0),
                    stop=(cj == NCH - 1),
                )
            nc.vector.tensor_add(r(bpv_row[:]), ps[:], bp_row[:])
            # WVP[d,o] = sum_c wv[c,d] wpT[c,o] (6 live banks, per wv chunk)
            wvps = [sc_ps([128, C]) for _ in range(2)]
            wvps += [out_ps([128, C]) for _ in range(3)]
            wvps += [rep_ps([128, C])]
            for cj in range(NCH):
                for di in range(NDH):
                    nc.tensor.matmul(
                        wvps[di][:],
                        lhsT=r(wv_nat[:, cj * D + di * 128 : cj * D + (di + 1) * 128]),
                        rhs=r(wpT[:, cj * C : (cj + 1) * C]),
                        start=(cj == 0),
                        stop=(cj == NCH - 1),
                    )
            for di in range(NDH):
                if di % 2 == 0:
                    nc.scalar.copy(r(WVP[:, di * C : (di + 1) * C]), wvps[di][:])
                else:
                    nc.vector.tensor_copy(r(WVP[:, di * C : (di + 1) * C]), wvps[di][:])

        # ---------------- main loop ----------------
        for bi, b in enumerate(batch_seq):
            xb = xs[bi]
            Ra, t2col, W2 = Ras.pop(bi), t2s.pop(bi), W2s.pop(bi)

            for sh in range(2):
                # scores S^T[m, s-half]; exp with per-partition bias
                sps_l = []
                for mj in range(NMH):
                    sps = sc_ps([128, 512])
                    for cj in range(NCH):
                        nc.tensor.matmul(
                            sps[:],
                            lhsT=r(Ra[:, cj * M + mj * 128 : cj * M + (mj + 1) * 128]),
                            rhs=r(xb[:, cj * S + sh * 512 : cj * S + (sh + 1) * 512]),
                            start=(cj == 0),
                            stop=(cj == NCH - 1),
                        )
                    sps_l.append(sps)
                if t2col is None:  # batch 0: bias emitted after first scores
                    t2col = emit_t(yTs[0], Ra, abes[0][1])
                P_sb = []
                for mj in range(NMH):
                    P = ppool.tile([128, 512], F32, tag="P", name="P")
                    nc.scalar.activation(
                        r(P[:]), sps_l[mj][:], AF.Exp,
                        bias=t2col[:, mj : mj + 1], scale=ATT_SCALE,
                    )
                    P_sb.append(P)

                # PE filler A (covers exp latency)
                if sh == 0:
                    if bi >= 1 and bi + 1 < NB:
                        yTs[bi + 1] = alloc_yT()
                        emit_yT_part(yTs[bi + 1], ys[bi + 1], range(0, 4), out_ps, "dve")
                else:
                    if bi > 0 and bi + 1 < NB:
                        W2s[bi + 1] = w2pool.tile([128, NMH * C], F32, tag="W2", name="W2")
                        emit_W2_mj(W2s[bi + 1], yTs[bi + 1], 0, out_ps)

                # rs[s] = sum_m exp
                rsps = rep_ps([1, 512])
                for mj in range(NMH):
                    nc.tensor.matmul(
                        rsps[:],
                        lhsT=r(onescol_sb[:]),
                        rhs=r(P_sb[mj][:]),
                        start=(mj == 0),
                        stop=(mj == NMH - 1),
                    )
                rinv_row = spool.tile([1, 512], F32, tag="rinv", name="rinv")
                with nc.allow_low_precision(reason="rinv feeds f32r bcast matmul"):
                    nc.vector.reciprocal(r(rinv_row[:]), rsps[:])
                if sh == 1 and bi + 1 < NB:
                    mvs_[bi + 1] += emit_stats_p1(xs[bi + 1], range(3, NCH))
                    stat2s[bi + 1] = emit_stats_stat2(mvs_[bi + 1])

                # PE filler B (covers recip latency)
                if sh == 0:
                    if bi == 0:
                        emit_WVP()
                        emit_W2_mj(W2, yTs[bi], 0)
                        emit_W2_mj(W2, yTs[bi], 1)
                    elif bi + 1 < NB:
                        emit_yT_part(yTs[bi + 1], ys[bi + 1], range(4, NDH), out_ps, "dve")
                    if bi >= 1 and bi + 2 < NB:
                        ys[bi + 2] = load_y(batch_seq[bi + 2])
                        xs[bi + 2] = load_x(batch_seq[bi + 2])

                # rinv broadcast to 128 partitions; normalize P in place
                repp = rep_ps([128, 512])
                nc.tensor.matmul(
                    repp[:],
                    lhsT=r(ones_sb[:, 0:128]),
                    rhs=r(rinv_row[:]),
                    start=True,
                    stop=True,
                )
                for mj in range(NMH):
                    nc.vector.tensor_mul(r(P_sb[mj][:]), P_sb[mj][:], repp[:])

                # out^T[o, s-half] = W2'^T Phat + x (identity matmul); copy; store
                nq = 1  # final-drain split disabled (regressed in sim)
                qw = 512 // nq
                for oj in range(NCH):
                    for q in range(nq):
                        ops_ = out_ps([128, qw])
                        for mj in range(NMH):
                            nc.tensor.matmul(
                                ops_[:],
                                lhsT=r(W2[:, mj * C + oj * 128 : mj * C + (oj + 1) * 128]),
                                rhs=r(P_sb[mj][:, q * qw : (q + 1) * qw]),
                                start=(mj == 0),
                                stop=False,
                            )
                        c0 = oj * S + sh * 512 + q * qw
                        nc.tensor.matmul(
                            ops_[:],
                            lhsT=r(eye_r[:]),
                            rhs=r(xb[:, c0 : c0 + qw]),
                            start=False,
                            stop=True,
                        )
                        ot = otpool.tile([128, qw], F32, tag="ot", name="ot")
                        if nq == 2 and (oj * nq + q) % 2 == 1:
                            nc.vector.tensor_copy(ot[:], ops_[:])
                        else:
                            nc.scalar.copy(ot[:], ops_[:])
                        nc.sync.dma_start(
                            out_d[b, oj * 128 : (oj + 1) * 128,
                                  sh * 512 + q * qw : sh * 512 + (q + 1) * qw],
                            ot[:],
                        )

                # next-batch GN stats (DVE), placed so the latency-critical
                # recip/Phat ops are never blocked behind them
                if bi + 1 < NB and sh == 0:
                    mvs_[bi + 1] = emit_stats_p1(xs[bi + 1], range(0, 3))

            # ---- batch tail: next-batch head work ----
            if bi + 1 < NB:
                bstats[bi + 1] = emit_stats_gnmm(stat2s.pop(bi + 1))
                if bi == 0:
                    W2s[1] = w2pool.tile([128, NMH * C], F32, tag="W2", name="W2")
                    emit_W2_mj(W2s[1], yTs[1], 0, out_ps)
                emit_W2_mj(W2s[bi + 1], yTs[bi + 1], 1, out_ps)  # covers Newton on DVE
                abes[bi + 1] = emit_stats_chan(bstats.pop(bi + 1))
                a_col, e_col = abes[bi + 1]
                Ranew = rapool.tile([128, NCH * M], F32, tag="Ra", name="Ra")
                for cj in range(NCH):
                    ps = emit_R_mm(yTs[bi + 1], cj, [sc_ps, sc_ps, out_ps, out_ps][cj])
                    nc.vector.tensor_scalar_mul(
                        r(Ranew[:, cj * M : (cj + 1) * M]), ps[:], a_col[:, cj : cj + 1]
                    )
                Ras[bi + 1] = Ranew
                t2s[bi + 1] = emit_t(yTs[bi + 1], Ranew, e_col)
            # drop refs for freed tiles
            for dd in (ys, xs, yTs, mvs_, abes):
                dd.pop(bi - 1, None)

    nc.compile()
    return nc


def make_const_inputs():
    gmap = np.zeros((C, G), np.float32)
    gmap[np.arange(C), np.arange(C) // CPG] = 1.0
    return {
        "eye": np.eye(128, dtype=np.float32),
        "ones": np.ones((1, S), np.float32),
        "onescol": np.ones((128, 1), np.float32),
        "gmap": gmap,
        "gmapT": np.ascontiguousarray(gmap.T),
    }


_CACHE = {}


def kernel(_trace=False, **inputs):
    if "nc" not in _CACHE:
        _CACHE["nc"] = build_program()
    nc = _CACHE["nc"]

    x = np.ascontiguousarray(inputs["x"], np.float32).reshape(B, C, S)
    y = np.ascontiguousarray(inputs["y"], np.float32)
    shared = {
        k: np.ascontiguousarray(inputs[k], np.float32)
        for k in ("wq", "wk", "wv", "wp", "bq", "bk", "bv", "bp", "gn_scale", "gn_bias")
    }
    shared.update(make_const_inputs())

    in_maps = []
    for i in range(NCORES):
        m = dict(shared)
        m["x"] = np.ascontiguousarray(x[i * BPC : (i + 1) * BPC])
        m["y"] = np.ascontiguousarray(y[i * BPC : (i + 1) * BPC])
        in_maps.append(m)

    from concourse.bass_utils import run_bass_kernel_spmd

    res = run_bass_kernel_spmd(nc, in_maps, list(range(NCORES)), trace=_trace)
    _CACHE["exec_time_ns"] = res.exec_time_ns
    _CACHE["result"] = res
    out = np.concatenate([res.results[i]["out"] for i in range(NCORES)], axis=0)
    return out.reshape(B, C, 32, 32)
